# revision 1
# baseline (speedup 1.0000x reference)
"""Sharded cosine-similarity kNN retrieval kernel for Trainium2 (Bass/Tile).

Problem: one query [D] against keys [N, D]; return actions[top_k indices of
cosine similarity].  N=100000, D=2048, A=7, top_k<=8.

Strategy:
  - Shard keys row-wise across 8 NeuronCores (12544 rows/core, last shard
    zero-padded).  Inputs are downcast to fp16 on the host: halves the HBM
    traffic and lets the DVE run 16-bit ops in 2x mode.  Selection is robust
    to fp16 rounding: top-8 similarity gaps (~1e-3) are ~100x larger than
    the fp16-induced sim error (~1e-5).
  - Per 128-row tile on each core:
      * dots[p]  = sum_d keys[p,d]*q[d]   via VectorE scalar_tensor_tensor
                   (fused multiply + free-dim accumulate, one pass)
      * norms2[p]= sum_d keys[p,d]^2      via ScalarE activation(Square,
                   accum_out=...) for 3 of 4 tiles, via a second VectorE
                   scalar_tensor_tensor for every 4th tile (engine balance:
                   DVE 1.22us + 0.25*1.22us vs ACT 0.75*2.0us per tile).
  - Host: sims = dots / max(|k| * |q|, eps), global top-k over 100k scalars,
    gather actions rows (the standard "reduce M*k candidates" step).
"""

import sys

for _p in ("/opt/trn_rl_repo", "/opt/trn_rl_repo/concourse"):
    if _p not in sys.path:
        sys.path.insert(0, _p)

import numpy as np

import concourse.bacc as bacc
from concourse import mybir
from concourse.bass_utils import run_bass_kernel_spmd
from concourse.tile import TileContext

N, D, A = 100000, 2048, 7
EPS = 1e-8
N_CORES = 8
P = 128
ROWS_PER_CORE = 12544            # 98 tiles of 128 rows; 8*12544 = 100352 >= N
TILES = ROWS_PER_CORE // P       # 98
DMA_CHUNK = 2                    # row-tiles per dma_start
DVE_SQ_MOD = 0                   # 0: all squares on ScalarE (DVE is the
                                 # bottleneck at 2.26us/tile; ACT 2.17us)
USE_FP16 = True

_CACHE = {}


def _build_bass(repeats: int = 1, fp16: bool = USE_FP16,
                dve_sq_mod: int = DVE_SQ_MOD, dma_chunk: int = DMA_CHUNK):
    """Build the per-core Bass program.

    repeats>1 wraps the streaming loop in a hardware For loop that re-reads
    the same DRAM shard; used only for wall-clock HW timing (slope over
    repeats cancels host/axon dispatch overhead)."""
    nc = bacc.Bacc(
        "TRN2",
        target_bir_lowering=False,
        debug=False,
        enable_asserts=False,
        num_devices=N_CORES,
    )
    f32 = mybir.dt.float32
    kdt = mybir.dt.float16 if fp16 else f32
    keys_d = nc.dram_tensor(
        "keys", [ROWS_PER_CORE, D], kdt, kind="ExternalInput"
    ).ap()
    qb_d = nc.dram_tensor("qb", [P, D], kdt, kind="ExternalInput").ap()
    dots_d = nc.dram_tensor(
        "dots", [P, TILES], f32, kind="ExternalOutput"
    ).ap()
    norms2_d = nc.dram_tensor(
        "norms2", [P, TILES], f32, kind="ExternalOutput"
    ).ap()

    # keys viewed as [p, t, d]: row t*128+p  ->  partition p, tile t
    keys_r = keys_d.rearrange("(t p) d -> p t d", p=P)

    with TileContext(nc) as tc:
        with tc.tile_pool(name="kpool", bufs=4) as kpool, \
             tc.tile_pool(name="spool", bufs=2) as spool, \
             tc.tile_pool(name="cpool", bufs=1) as cpool:
            qb_t = cpool.tile([P, D], kdt)
            nc.sync.dma_start(out=qb_t, in_=qb_d)
            dots_t = cpool.tile([P, TILES], f32)
            norms_t = cpool.tile([P, TILES], f32)

            def body():
                for c in range(TILES // dma_chunk):
                    kt = kpool.tile([P, dma_chunk, D], kdt, tag="keys",
                                    name="kt")
                    nc.sync.dma_start(
                        out=kt,
                        in_=keys_r[:, c * dma_chunk:(c + 1) * dma_chunk, :],
                    )
                    for j in range(dma_chunk):
                        t = c * dma_chunk + j
                        prod = spool.tile([P, D], kdt, tag="prod", name="prod")
                        if fp16 and t % 33 == 16:
                            # engine balance: DVE's fused dot (1x, 2.26us)
                            # is the kernel bottleneck; for 3 of 98 tiles
                            # do the multiply at 2x on DVE and let ScalarE
                            # (which has slack) do the reduction.
                            nc.vector.tensor_mul(prod, kt[:, j, :], qb_t)
                            cout = spool.tile([P, D], kdt, tag="cout",
                                              name="cout")
                            nc.scalar.activation(
                                cout,
                                prod,
                                mybir.ActivationFunctionType.Copy,
                                accum_out=dots_t[:, t:t + 1],
                            )
                        else:
                            nc.vector.scalar_tensor_tensor(
                                out=prod,
                                in0=kt[:, j, :],
                                scalar=1.0,
                                in1=qb_t,
                                op0=mybir.AluOpType.bypass,
                                op1=mybir.AluOpType.mult,
                                accum_out=dots_t[:, t:t + 1],
                            )
                        sq = spool.tile([P, D], kdt, tag="sq", name="sq")
                        if dve_sq_mod and t % dve_sq_mod == 0:
                            nc.vector.scalar_tensor_tensor(
                                out=sq,
                                in0=kt[:, j, :],
                                scalar=1.0,
                                in1=kt[:, j, :],
                                op0=mybir.AluOpType.bypass,
                                op1=mybir.AluOpType.mult,
                                accum_out=norms_t[:, t:t + 1],
                            )
                        else:
                            nc.scalar.activation(
                                sq,
                                kt[:, j, :],
                                mybir.ActivationFunctionType.Square,
                                accum_out=norms_t[:, t:t + 1],
                            )

            if repeats == 1:
                body()
            else:
                with tc.For_i(0, repeats, 1):
                    body()

            nc.sync.dma_start(out=dots_d, in_=dots_t)
            nc.sync.dma_start(out=norms2_d, in_=norms_t)
    nc.compile()
    return nc


def _get_nc(repeats: int = 1, **kw):
    key = ("nc", repeats, tuple(sorted(kw.items())))
    if key not in _CACHE:
        _CACHE[key] = _build_bass(repeats, **kw)
    return _CACHE[key]


def _make_in_maps(keys: np.ndarray, query: np.ndarray,
                  fp16: bool = USE_FP16):
    dt = np.float16 if fp16 else np.float32
    qb = np.ascontiguousarray(
        np.broadcast_to(query.astype(dt), (P, D))
    )
    in_maps = []
    for i in range(N_CORES):
        lo, hi = i * ROWS_PER_CORE, (i + 1) * ROWS_PER_CORE
        if hi <= N:
            shard = np.ascontiguousarray(keys[lo:hi], dtype=dt)
        else:
            shard = np.zeros((ROWS_PER_CORE, D), dtype=dt)
            shard[: N - lo] = keys[lo:N].astype(dt)
        in_maps.append({"keys": shard, "qb": qb})
    return in_maps


def _run_device(keys: np.ndarray, query: np.ndarray, trace: bool = False):
    """Run the SPMD kernel; returns (dots[100352], norms2[100352], results)."""
    nc = _get_nc()
    in_maps = _make_in_maps(keys, query)
    res = run_bass_kernel_spmd(
        nc, in_maps, core_ids=list(range(N_CORES)), trace=trace
    )
    dots = np.empty(N_CORES * ROWS_PER_CORE, np.float32)
    norms2 = np.empty(N_CORES * ROWS_PER_CORE, np.float32)
    for i, out in enumerate(res.results):
        # out["dots"][p, t] is row t*128+p of shard i
        base = i * ROWS_PER_CORE
        dots[base:base + ROWS_PER_CORE] = out["dots"].T.reshape(-1)
        norms2[base:base + ROWS_PER_CORE] = out["norms2"].T.reshape(-1)
    return dots, norms2, res


def kernel(**inputs) -> np.ndarray:
    query = np.asarray(inputs["query_key"], dtype=np.float32)
    keys = np.asarray(inputs["keys"], dtype=np.float32)
    actions = np.asarray(inputs["actions"])
    top_k = int(inputs["top_k"])
    if top_k <= 0:
        return actions[:0]
    top_k = min(top_k, keys.shape[0])

    dots, norms2, _ = _run_device(keys, query)
    dots = dots[:N]
    norms2 = norms2[:N]

    q16 = query.astype(np.float16).astype(np.float32)
    q_norm = np.float32(np.linalg.norm(q16))
    denom = np.maximum(np.sqrt(norms2) * q_norm, np.float32(EPS))
    sims = dots / denom

    # top_k, ties resolved to the lower index (jax.lax.top_k semantics)
    cand = np.argpartition(-sims, top_k - 1)[:top_k]
    order = np.lexsort((cand, -sims[cand]))
    idx = cand[order]
    return actions[idx]



# revision 2
# speedup vs baseline: 2.7791x; 2.7791x over previous
"""Sharded cosine-similarity kNN retrieval kernel for Trainium2 (Bass/Tile).

Problem: one query [D] against keys [N, D]; return actions[top_k indices of
cosine similarity].  N=100000, D=2048, A=7, top_k<=8.

Strategy (v2 — TensorE fp8 matvec, ~3x over the fp16 DVE/ACT baseline):
  - Keys are cast to fp8(e4m3, TRN flavor: ml_dtypes.float8_e4m3) and
    pre-transposed on the host into a d-major layout, sharded row-wise
    across 8 cores (12800 rows/core, zero-padded).  HBM traffic is
    25.6 MB/core; at ~358 GB/s/NC the DMA floor is ~72 us, which is the
    kernel's design bottleneck (the fp16 DVE/ACT baseline was
    compute-bound at ~223 us).
  - All dot products run on the otherwise-idle TensorE with the query
    stationary: lhsT = q-chunk [128, 2, 1] (LDWEIGHTS of a 1-column
    weight is ~free), rhs = keysT block [128, 2, 256] streaming in fp8
    DoubleRow perf mode (2 fp8 MACs/cell/cycle, K=256 per pass).  Eight
    chunk-matmuls accumulate the full D=2048 contraction into PSUM
    [1, 256] fp32.  PE time ~50 us/core < DMA ~73 us.
  - Per core the layout is [10 super-blocks, 128, 8, 2, 1280]: one
    2.62 MB contiguous-per-partition DMA per super-block (20.5 KB
    lines), triple-buffered so the PE never idles a full HAM window.
  - DVE only copies PSUM->SBUF dots; a single 51 KB DMA returns
    dots[12800] fp32 per core.
  - Host reduction (standard sharded-kNN candidate step): top-64 rows
    per core by raw device dot (norm spread is 20+ sigma below the
    rank-64 margin), then an exact fp64 rescore of the 512 candidates
    reproduces the reference ordering: sims = dot / max(|k||q|, eps),
    top_k with ties to the lower index.
"""

import sys

for _p in ("/opt/trn_rl_repo", "/opt/trn_rl_repo/concourse"):
    if _p not in sys.path:
        sys.path.insert(0, _p)

import numpy as np

import concourse.bacc as bacc
from concourse import mybir
from concourse.bass_utils import run_bass_kernel_spmd
from concourse.tile import TileContext

N, D, A = 100000, 2048, 7
EPS = 1e-8
N_CORES = 8
P = 128
PAIR = 2                       # DoubleRow k-tile pair
CHUNKS = D // (P * PAIR)       # 8 contraction super-chunks of 256
SB = 10                        # super-blocks per core
RB = 1280                      # rows per super-block
NB = RB // 256                 # 5 matmul col-blocks of 256 rows
ROWS_PER_CORE = SB * RB        # 12800; 8*12800 = 102400 >= N
CAND_PER_CORE = 64             # candidates kept per core before exact rescore
USE_DOUBLE_ROW = True

F8 = np.dtype(mybir.dt.np(mybir.dt.float8e4))

_CACHE = {}


def _build_bass(repeats: int = 1, double_row: bool = USE_DOUBLE_ROW):
    """Build the per-core Bass program.

    repeats>1 wraps the streaming loop in a hardware For loop that re-reads
    the same DRAM shard; used only for wall-clock HW timing (slope over
    repeats cancels host/axon dispatch overhead)."""
    nc = bacc.Bacc(
        "TRN2",
        target_bir_lowering=False,
        debug=False,
        enable_asserts=False,
        num_devices=N_CORES,
    )
    f32 = mybir.dt.float32
    f8 = mybir.dt.float8e4
    # keysr[sb, p, c, i, r] = fp8(keys[sb*RB + r, c*256 + i*128 + p])
    keys_d = nc.dram_tensor(
        "keysr", [SB, P, CHUNKS, PAIR, RB], f8, kind="ExternalInput"
    ).ap()
    # qw[p, c, i, 0] = fp8(q[c*256 + i*128 + p]); 16-wide so the DoubleRow
    # weight AP's pair-stride is 16 B (hardware requires step % 16 == 0)
    qw_d = nc.dram_tensor("qw", [P, CHUNKS, PAIR, 16], f8, kind="ExternalInput").ap()
    dots_d = nc.dram_tensor("dots", [1, ROWS_PER_CORE], f32, kind="ExternalOutput").ap()

    mm_mode = mybir.MatmulPerfMode.DoubleRow if double_row else None

    with TileContext(nc) as tc:
        with tc.tile_pool(name="kpool", bufs=3) as kpool, \
             tc.tile_pool(name="ppool", bufs=2, space="PSUM") as ppool, \
             tc.tile_pool(name="cpool", bufs=1) as cpool:
            qw_t = cpool.tile([P, CHUNKS, PAIR, 16], f8)
            nc.sync.dma_start(out=qw_t, in_=qw_d)
            dots_t = cpool.tile([1, ROWS_PER_CORE], f32)

            def body():
                for sb in range(SB):
                    kt = kpool.tile([P, CHUNKS, PAIR, RB], f8, tag="kt", name="kt")
                    nc.sync.dma_start(out=kt, in_=keys_d[sb])
                    ps = ppool.tile([1, RB], f32, tag="ps", name="ps")
                    for nb in range(NB):
                        sl = slice(nb * 256, (nb + 1) * 256)
                        if double_row:
                            for c in range(CHUNKS):
                                nc.tensor.matmul(
                                    ps[:, sl],
                                    lhsT=qw_t[:, c, :, 0:1],
                                    rhs=kt[:, c, :, sl],
                                    start=(c == 0),
                                    stop=(c == CHUNKS - 1),
                                    perf_mode=mm_mode,
                                )
                        else:
                            for c in range(CHUNKS):
                                for i in range(PAIR):
                                    nc.tensor.matmul(
                                        ps[:, sl],
                                        lhsT=qw_t[:, c, i, 0:1],
                                        rhs=kt[:, c, i, sl],
                                        start=(c == 0 and i == 0),
                                        stop=(c == CHUNKS - 1 and i == PAIR - 1),
                                    )
                    nc.vector.tensor_copy(dots_t[:, sb * RB:(sb + 1) * RB], ps)

            if repeats == 1:
                body()
            else:
                with tc.For_i(0, repeats, 1):
                    body()

            nc.sync.dma_start(out=dots_d, in_=dots_t)
    nc.compile()
    return nc


def _get_nc(repeats: int = 1, **kw):
    key = ("nc", repeats, tuple(sorted(kw.items())))
    if key not in _CACHE:
        _CACHE[key] = _build_bass(repeats, **kw)
    return _CACHE[key]


def _pack_keys_shard(shard_f32: np.ndarray) -> np.ndarray:
    """[ROWS_PER_CORE, D] fp32 -> [SB, P, CHUNKS, PAIR, RB] fp8."""
    s8 = shard_f32.astype(F8)
    v = s8.reshape(SB, RB, CHUNKS, PAIR, P).transpose(0, 4, 2, 3, 1)
    return np.ascontiguousarray(v)


def _make_in_maps(keys: np.ndarray, query: np.ndarray):
    q8 = query.astype(np.float32).astype(F8)
    qw = np.zeros((P, CHUNKS, PAIR, 16), dtype=F8)
    qw[:, :, :, 0] = q8.reshape(CHUNKS, PAIR, P).transpose(2, 0, 1)
    in_maps = []
    for i in range(N_CORES):
        lo, hi = i * ROWS_PER_CORE, (i + 1) * ROWS_PER_CORE
        if hi <= N:
            shard = keys[lo:hi]
        else:
            shard = np.zeros((ROWS_PER_CORE, D), dtype=np.float32)
            if lo < N:
                shard[: N - lo] = keys[lo:N]
        in_maps.append({"keysr": _pack_keys_shard(shard), "qw": qw})
    return in_maps


def _run_device(keys: np.ndarray, query: np.ndarray, trace: bool = False):
    """Run the SPMD kernel; returns (dots[8*ROWS_PER_CORE] fp32, results)."""
    nc = _get_nc()
    in_maps = _make_in_maps(keys, query)
    res = run_bass_kernel_spmd(
        nc, in_maps, core_ids=list(range(N_CORES)), trace=trace
    )
    dots = np.empty(N_CORES * ROWS_PER_CORE, np.float32)
    for i, out in enumerate(res.results):
        base = i * ROWS_PER_CORE
        dots[base:base + ROWS_PER_CORE] = out["dots"].reshape(-1)
    return dots, res


def kernel(**inputs) -> np.ndarray:
    query = np.asarray(inputs["query_key"], dtype=np.float32)
    keys = np.asarray(inputs["keys"], dtype=np.float32)
    actions = np.asarray(inputs["actions"])
    top_k = int(inputs["top_k"])
    if top_k <= 0:
        return actions[:0]
    top_k = min(top_k, keys.shape[0])

    dots, _ = _run_device(keys, query)

    # candidate reduction: top-C per core by raw fp8 dot
    cand_per_core = max(CAND_PER_CORE, 4 * top_k)
    cands = []
    for i in range(N_CORES):
        lo = i * ROWS_PER_CORE
        n_valid = min(N - lo, ROWS_PER_CORE)
        if n_valid <= 0:
            continue
        d = dots[lo:lo + n_valid]
        c = min(cand_per_core, n_valid)
        top = np.argpartition(-d, c - 1)[:c] + lo
        cands.append(top)
    cand = np.concatenate(cands)

    # exact rescore of candidates (fp64), jax.lax.top_k tie semantics
    q64 = query.astype(np.float64)
    kc = keys[cand].astype(np.float64)
    sims = (kc @ q64) / np.maximum(
        np.linalg.norm(kc, axis=1) * np.linalg.norm(q64), EPS
    )
    order = np.lexsort((cand, -sims))
    idx = cand[order[:top_k]]
    return actions[idx]


# revision 5
# speedup vs baseline: 3.5303x; 1.2703x over previous
"""Sharded cosine-similarity kNN retrieval kernel for Trainium2 (Bass/Tile).

Problem: one query [D] against keys [N, D]; return actions[top_k indices of
cosine similarity].  N=100000, D=2048, A=7, top_k<=8.

Strategy (v3 — TensorE fp8 matvec, ~3.1x over the fp16 DVE/ACT baseline):
  - Keys are cast to fp8(e4m3, TRN flavor: ml_dtypes.float8_e4m3) and
    pre-transposed on the host into a d-major layout, sharded row-wise
    across 8 cores (12544 rows/core, zero-padded).  HBM traffic is
    25.7 MB/core; measured DMA rate ~354 GB/s sets the kernel time
    (the fp16 DVE/ACT baseline was compute-bound at ~223 us).
  - All dot products run on the otherwise-idle TensorE with the query
    stationary: lhsT = q-chunk [128, 2, 1] (LDWEIGHTS of a 1-column
    weight is ~free), rhs = keysT block [128, 2, 256] streaming in fp8
    DoubleRow perf mode (2 fp8 MACs/cell/cycle, K=256 per pass).  Eight
    chunk-matmuls accumulate the full D=2048 contraction into PSUM
    [1, 256] fp32.  PE time ~50 us/core < DMA ~72 us.
  - Per core the layout is [49 blocks, 128, 8, 2, 256]: one 0.52 MB
    contiguous-per-partition DMA per block, alternated between the two
    HWDGE rings (nc.sync=SP, nc.scalar=ACT) with 6 SBUF buffers.
    Empirically many small dual-ring DMAs sustain ~354 GB/s vs
    ~315 GB/s for few large single-ring ones.
  - DVE only copies PSUM->SBUF dots; a single 50 KB DMA returns
    dots[12544] fp32 per core.
  - Host reduction (standard sharded-kNN candidate step): top-64 rows
    per core by raw device dot (norm spread is 20+ sigma below the
    rank-64 margin; measured worst true-top-9 rank is 3), then an exact
    fp64 rescore of the 512 candidates reproduces the reference
    ordering: sims = dot / max(|k||q|, eps), top_k ties to lower index.
"""

import sys

for _p in ("/opt/trn_rl_repo", "/opt/trn_rl_repo/concourse"):
    if _p not in sys.path:
        sys.path.insert(0, _p)

import numpy as np

import concourse.bacc as bacc
from concourse import mybir
from concourse.bass_utils import run_bass_kernel_spmd
from concourse.tile import TileContext

N, D, A = 100000, 2048, 7
EPS = 1e-8
N_CORES = 8
P = 128
PAIR = 2                       # DoubleRow k-tile pair
CHUNKS = D // (P * PAIR)       # 8 contraction super-chunks of 256
SB = 49                        # blocks per core
RB = 256                       # rows per block
ROWS_PER_CORE = SB * RB        # 12544; 8*12544 = 100352 >= N
KBUFS = 6                      # SBUF key-tile buffers (DMA prefetch depth)
UNROLL = 8                     # bodies per hardware-loop iteration (timing)
CAND_PER_CORE = 128            # candidates kept per core before exact rescore
USE_DOUBLE_ROW = True

F8 = np.dtype(mybir.dt.np(mybir.dt.float8e4))

_CACHE = {}


def _build_bass(repeats: int = 1, double_row: bool = USE_DOUBLE_ROW):
    """Build the per-core Bass program.

    repeats>1 wraps the streaming loop in a hardware For loop that re-reads
    the same DRAM shard; used only for wall-clock HW timing (slope over
    repeats cancels host/axon dispatch overhead)."""
    nc = bacc.Bacc(
        "TRN2",
        target_bir_lowering=False,
        debug=False,
        enable_asserts=False,
        num_devices=N_CORES,
    )
    f32 = mybir.dt.float32
    f8 = mybir.dt.float8e4
    # keysr[sb, p, c, i, r] = fp8(keys[sb*RB + r, c*256 + i*128 + p])
    keys_d = nc.dram_tensor(
        "keysr", [SB, P, CHUNKS, PAIR, RB], f8, kind="ExternalInput"
    ).ap()
    # qw[p, c, i, 0] = fp8(q[c*256 + i*128 + p]); 16-wide so the DoubleRow
    # weight AP's pair-stride is 16 B (hardware requires step % 16 == 0)
    qw_d = nc.dram_tensor("qw", [P, CHUNKS, PAIR, 16], f8, kind="ExternalInput").ap()
    dots_d = nc.dram_tensor("dots", [1, ROWS_PER_CORE], f32, kind="ExternalOutput").ap()

    mm_mode = mybir.MatmulPerfMode.DoubleRow if double_row else None

    with TileContext(nc) as tc:
        with tc.tile_pool(name="kpool", bufs=KBUFS) as kpool, \
             tc.tile_pool(name="ppool", bufs=2, space="PSUM") as ppool, \
             tc.tile_pool(name="cpool", bufs=1) as cpool:
            qw_t = cpool.tile([P, CHUNKS, PAIR, 16], f8)
            nc.sync.dma_start(out=qw_t, in_=qw_d)
            dots_t = cpool.tile([1, ROWS_PER_CORE], f32)

            def body():
                for sb in range(SB):
                    kt = kpool.tile([P, CHUNKS, PAIR, RB], f8, tag="kt", name="kt")
                    # alternate between the two HWDGE rings (SP / ACT)
                    eng = nc.scalar if sb % 2 else nc.sync
                    eng.dma_start(out=kt, in_=keys_d[sb])
                    ps = ppool.tile([1, RB], f32, tag="ps", name="ps")
                    if double_row:
                        for c in range(CHUNKS):
                            nc.tensor.matmul(
                                ps,
                                lhsT=qw_t[:, c, :, 0:1],
                                rhs=kt[:, c, :, :],
                                start=(c == 0),
                                stop=(c == CHUNKS - 1),
                                perf_mode=mm_mode,
                            )
                    else:
                        for c in range(CHUNKS):
                            for i in range(PAIR):
                                nc.tensor.matmul(
                                    ps,
                                    lhsT=qw_t[:, c, i, 0:1],
                                    rhs=kt[:, c, i, :],
                                    start=(c == 0 and i == 0),
                                    stop=(c == CHUNKS - 1 and i == PAIR - 1),
                                )
                    nc.vector.tensor_copy(dots_t[:, sb * RB:(sb + 1) * RB], ps)

            if repeats == 1:
                body()
            else:
                unroll = UNROLL if repeats % UNROLL == 0 else 1
                with tc.For_i(0, repeats // unroll, 1):
                    for _ in range(unroll):
                        body()

            nc.sync.dma_start(out=dots_d, in_=dots_t)
    nc.compile()
    return nc


def _get_nc(repeats: int = 1, **kw):
    key = ("nc", repeats, tuple(sorted(kw.items())))
    if key not in _CACHE:
        _CACHE[key] = _build_bass(repeats, **kw)
    return _CACHE[key]


def _pack_keys_shard(shard_f32: np.ndarray) -> np.ndarray:
    """[ROWS_PER_CORE, D] fp32 -> [SB, P, CHUNKS, PAIR, RB] fp8."""
    s8 = shard_f32.astype(F8)
    v = s8.reshape(SB, RB, CHUNKS, PAIR, P).transpose(0, 4, 2, 3, 1)
    return np.ascontiguousarray(v)


def _make_in_maps(keys: np.ndarray, query: np.ndarray):
    q8 = query.astype(np.float32).astype(F8)
    qw = np.zeros((P, CHUNKS, PAIR, 16), dtype=F8)
    qw[:, :, :, 0] = q8.reshape(CHUNKS, PAIR, P).transpose(2, 0, 1)
    in_maps = []
    for i in range(N_CORES):
        lo, hi = i * ROWS_PER_CORE, (i + 1) * ROWS_PER_CORE
        if hi <= N:
            shard = keys[lo:hi]
        else:
            shard = np.zeros((ROWS_PER_CORE, D), dtype=np.float32)
            if lo < N:
                shard[: N - lo] = keys[lo:N]
        in_maps.append({"keysr": _pack_keys_shard(shard), "qw": qw})
    return in_maps


def _run_device(keys: np.ndarray, query: np.ndarray, trace: bool = False):
    """Run the SPMD kernel; returns (dots[8*ROWS_PER_CORE] fp32, results)."""
    nc = _get_nc()
    in_maps = _make_in_maps(keys, query)
    res = run_bass_kernel_spmd(
        nc, in_maps, core_ids=list(range(N_CORES)), trace=trace
    )
    dots = np.empty(N_CORES * ROWS_PER_CORE, np.float32)
    for i, out in enumerate(res.results):
        base = i * ROWS_PER_CORE
        dots[base:base + ROWS_PER_CORE] = out["dots"].reshape(-1)
    return dots, res


def kernel(**inputs) -> np.ndarray:
    query = np.asarray(inputs["query_key"], dtype=np.float32)
    keys = np.asarray(inputs["keys"], dtype=np.float32)
    actions = np.asarray(inputs["actions"])
    top_k = int(inputs["top_k"])
    if top_k <= 0:
        return actions[:0]
    top_k = min(top_k, keys.shape[0])

    dots, _ = _run_device(keys, query)

    # candidate reduction: top-C per core by raw fp8 dot
    cand_per_core = max(CAND_PER_CORE, 4 * top_k)
    cands = []
    for i in range(N_CORES):
        lo = i * ROWS_PER_CORE
        n_valid = min(N - lo, ROWS_PER_CORE)
        if n_valid <= 0:
            continue
        d = dots[lo:lo + n_valid]
        c = min(cand_per_core, n_valid)
        top = np.argpartition(-d, c - 1)[:c] + lo
        cands.append(top)
    cand = np.concatenate(cands)

    # exact rescore of candidates (fp64), jax.lax.top_k tie semantics
    q64 = query.astype(np.float64)
    kc = keys[cand].astype(np.float64)
    sims = (kc @ q64) / np.maximum(
        np.linalg.norm(kc, axis=1) * np.linalg.norm(q64), EPS
    )
    order = np.lexsort((cand, -sims))
    idx = cand[order[:top_k]]
    return actions[idx]
